# revision 1
# baseline (speedup 1.0000x reference)
"""MetaGAT Trainium2 kernel (8 NeuronCores, SPMD).

Strategy (batch-filtered slot grid, host-staged dense edge data):
  The output only depends on h_u rows at the batch user ids `u` (and h_i at
  `i`).  Each core takes a 2048-slice of the batch; the host selects the
  edges whose destination is in that slice's id set (~20K of 2M edges per
  core per side) and builds a degree-bucketed slot grid (dst -> partition
  lane, edge slot -> grid column).  Instead of on-device gathers, the host
  stages the grid's source-embedding rows DENSELY in grid order as one bf16
  tensor [128, SL, 64] per side, so the device does contiguous DMAs per side
  (no per-edge indirection, no SWDGE overhead).  All model arithmetic runs
  on-device:
    - edge scores  s_src = F * wa  via whole-grid DVE mult + reduce (halved
      for DMA/compute overlap)
    - e = lrelu(s_src + s_dst) on the ACT engine (bias = per-lane s_dst,
      alpha = leak) per window; additive -60 mask kills padding slots
    - exp + per-window softmax denominator in one ACT op (accum_out)
    - unnormalized weighted sums via whole-grid mult (output transposed
      [128, d, SL] bf16) + per-window contiguous reduces; normalization by
      recip(den) folded into the final per-window scale
    - batch phase: fused [dst_emb | h] scratch in DRAM, gathered per batch
      element with two 1024-idx dma_gathers on dedicated queues, transposed
      via PE, then the three Linear layers on PE.
  Outputs are [64, 2048] transposed slices per side per core; the host
  reassembles the [16384, 128] result.
"""
import numpy as np
import ml_dtypes

EMB = 64
NNODE = 200000
NCORES = 8
B = 16384
BC = B // NCORES          # 2048 batch rows per core
GPAD = BC                 # grid slots (>= unique dst count, <= BC)
NW = GPAD // 128          # 16 windows of 128 dst lanes
F32MIN = 1e-30
SB = 1280                 # padded batch-stream length (per rank-half)
MASKNEG = -60.0           # added after lrelu; exp(-60) ~ 9e-27


# ----------------------------------------------------------------- host prep

def _wrap16(idx):
    """dma_gather idx layout: j -> [j%16, j//16], replicated for 8 Q7 cores."""
    a = np.ascontiguousarray(idx.reshape(-1, 16).T)
    return np.tile(a, (8, 1))


def _prep_core_side(bat_c, src_ids, dst_ids):
    """Pure index bookkeeping for one (core, side): select + grid-order edges."""
    uniq, inv = np.unique(bat_c, return_inverse=True)
    G = uniq.size
    lut = np.full(NNODE, -1, np.int32)
    lut[uniq] = np.arange(G, dtype=np.int32)
    eg = lut[dst_ids]
    m = eg >= 0
    es = src_ids[m].astype(np.int64)
    eg = eg[m].astype(np.int64)
    deg = np.bincount(eg, minlength=G)
    order = np.argsort(-deg, kind="stable")          # grid rank -> uniq idx
    pos = np.empty(G, np.int64)
    pos[order] = np.arange(G)
    deg_r = deg[order]                               # degree by rank (desc)
    ep = pos[eg]                                     # edge -> grid rank
    eo = np.argsort(ep, kind="stable")
    es_s = es[eo]                                    # edge -> src node id
    ep_s = ep[eo]
    starts = np.zeros(G + 1, np.int64)
    np.cumsum(deg_r, out=starts[1:])
    ii = np.arange(es_s.size) - starts[ep_s]         # slot index within dst
    bslot = pos[inv]                                 # batch row -> grid rank
    return dict(G=G, uniq=uniq, order=order, deg_r=deg_r, es_s=es_s,
                ep_s=ep_s, ii=ii, bslot=bslot)


def _streams(pc, KS, emb_src, emb_dst):
    """Build device arrays for one (core, side) under the common schedule KS."""
    SL = sum(KS)
    CW = np.concatenate([[0], np.cumsum(KS)]).astype(np.int64)
    G = pc["G"]
    lane = pc["ep_s"] % 128
    win = pc["ep_s"] // 128
    col = CW[win] + pc["ii"]
    F = np.zeros((128, SL, EMB), np.float32)
    F[lane, col, :] = emb_src[pc["es_s"]]
    maskneg = np.full((128, SL), MASKNEG, np.float32)
    maskneg[lane, col] = 0.0
    dst_tab = np.zeros((GPAD, EMB), np.float32)
    dst_tab[:G] = emb_dst[pc["uniq"][pc["order"]]]
    dst_tab = dst_tab.astype(ml_dtypes.bfloat16)
    return dict(
        fgrid=np.ascontiguousarray(F.reshape(128, SL * EMB)).astype(
            ml_dtypes.bfloat16),
        fgridT=np.ascontiguousarray(
            F.transpose(2, 1, 0).reshape(EMB, SL * 128)).astype(
            ml_dtypes.bfloat16),
        maskneg=maskneg,
        dst_tab=dst_tab,
        bslot=_wrap16(pc["bslot"].astype(np.int16)),
    )


def _prep_all(inputs):
    u = np.asarray(inputs["u"]).astype(np.int64)
    i_ = np.asarray(inputs["i"]).astype(np.int64)
    sides = {
        "u": dict(bat=u, src=np.asarray(inputs["src_iu"]).astype(np.int64),
                  dst=np.asarray(inputs["dst_iu"]).astype(np.int64),
                  emb_src=np.asarray(inputs["item_emb"], np.float32),
                  emb_dst=np.asarray(inputs["user_emb"], np.float32)),
        "i": dict(bat=i_, src=np.asarray(inputs["src_ui"]).astype(np.int64),
                  dst=np.asarray(inputs["dst_ui"]).astype(np.int64),
                  emb_src=np.asarray(inputs["user_emb"], np.float32),
                  emb_dst=np.asarray(inputs["item_emb"], np.float32)),
    }
    pcs = {s: [_prep_core_side(sides[s]["bat"][c * BC:(c + 1) * BC],
                               sides[s]["src"], sides[s]["dst"])
               for c in range(NCORES)] for s in sides}
    # common window schedule across cores & sides
    KS = []
    for w in range(NW):
        k = 1
        for s in pcs:
            for pc in pcs[s]:
                if w * 128 < pc["G"]:
                    k = max(k, int(pc["deg_r"][w * 128]))
        KS.append(k)
    for s in pcs:
        for pc in pcs[s]:
            assert int(pc["deg_r"][0]) <= KS[0]
    per_core = []
    for c in range(NCORES):
        d = {}
        for s in pcs:
            st = _streams(pcs[s][c], KS, sides[s]["emb_src"],
                          sides[s]["emb_dst"])
            for k, v in st.items():
                d[f"{k}_{s}"] = v
        per_core.append(d)
    # weights (identical on every core)
    w = {}
    for s, wa, Ws, bs, Wn, bn, Wfc in (
            ("u", inputs["Wa_u"], inputs["Ws_u"], inputs["bs_u"],
             inputs["Wn_u"], inputs["bn_u"], inputs["Wfc_u"]),
            ("i", inputs["Wa_i"], inputs["Ws_i"], inputs["bs_i"],
             inputs["Wn_i"], inputs["bn_i"], inputs["Wfc_i"])):
        wa = np.asarray(wa, np.float32)
        w[f"wa_src_{s}"] = np.ascontiguousarray(
            wa[:EMB].reshape(EMB, 1)).astype(ml_dtypes.bfloat16)
        w[f"wa_dst_{s}"] = np.tile(wa[EMB:][None, :], (128, 1)).astype(
            ml_dtypes.bfloat16)
        w[f"WsT_{s}"] = np.ascontiguousarray(
            np.asarray(Ws, np.float32).T).astype(ml_dtypes.bfloat16)
        w[f"WnT_{s}"] = np.ascontiguousarray(
            np.asarray(Wn, np.float32).T).astype(ml_dtypes.bfloat16)
        Wfc = np.asarray(Wfc, np.float32)
        w[f"WfcS_{s}"] = np.ascontiguousarray(
            Wfc[:, :EMB].T).astype(ml_dtypes.bfloat16)
        w[f"WfcN_{s}"] = np.ascontiguousarray(
            Wfc[:, EMB:].T).astype(ml_dtypes.bfloat16)
        w[f"bs_{s}"] = np.asarray(bs, np.float32).reshape(EMB, 1)
        w[f"bn_{s}"] = np.asarray(bn, np.float32).reshape(EMB, 1)
    for d in per_core:
        d.update(w)
    cfg = dict(KS=tuple(KS))
    return cfg, per_core


# ------------------------------------------------------------- device kernel

def _build_nc(cfg):
    import concourse.bacc as bacc
    import concourse.mybir as mybir
    import concourse.tile as tile
    from concourse.masks import make_identity
    from concourse.tile_rust import add_dep_helper

    f32 = mybir.dt.float32
    bf16 = mybir.dt.bfloat16
    fp8 = mybir.dt.float8e4
    i16 = mybir.dt.int16
    KS = cfg["KS"]
    SL = sum(KS)                      # slot-grid columns
    CW = np.concatenate([[0], np.cumsum(KS)]).astype(int)
    WH = NW // 2                      # first-half windows
    SH = int(CW[WH])                  # first-half columns
    Alu = mybir.AluOpType
    Act = mybir.ActivationFunctionType

    nc = bacc.Bacc("TRN2", num_swdge_queues=4)
    T = {}
    for s in ("u", "i"):
        T[f"fgrid_{s}"] = nc.dram_tensor(f"fgrid_{s}", [128, SL * EMB], bf16, kind="ExternalInput")
        T[f"fgridT_{s}"] = nc.dram_tensor(f"fgridT_{s}", [EMB, SL * 128], bf16, kind="ExternalInput")
        T[f"maskneg_{s}"] = nc.dram_tensor(f"maskneg_{s}", [128, SL], f32, kind="ExternalInput")
        T[f"dst_tab_{s}"] = nc.dram_tensor(f"dst_tab_{s}", [GPAD, EMB], bf16, kind="ExternalInput")
        T[f"bslot_{s}"] = nc.dram_tensor(f"bslot_{s}", [128, BC // 16], i16, kind="ExternalInput")
        T[f"wa_src_{s}"] = nc.dram_tensor(f"wa_src_{s}", [EMB, 1], bf16, kind="ExternalInput")
        T[f"wa_dst_{s}"] = nc.dram_tensor(f"wa_dst_{s}", [128, EMB], bf16, kind="ExternalInput")
        for nm in ("WsT", "WnT", "WfcS", "WfcN"):
            T[f"{nm}_{s}"] = nc.dram_tensor(f"{nm}_{s}", [EMB, EMB], bf16, kind="ExternalInput")
        for nm in ("bs", "bn"):
            T[f"{nm}_{s}"] = nc.dram_tensor(f"{nm}_{s}", [EMB, 1], f32, kind="ExternalInput")
        T[f"outT_{s}"] = nc.dram_tensor(f"outT_{s}", [EMB, BC], f32, kind="ExternalOutput")
        T[f"scratch_{s}"] = nc.dram_tensor(f"scratch_{s}", [GPAD, 2 * EMB], bf16, kind="Internal")


    with tile.TileContext(nc) as tc:
        with (
            tc.tile_pool(name="fpool", bufs=1) as fpool,      # big grids
            tc.tile_pool(name="lpool", bufs=1) as lpool,      # hoisted loads
            tc.tile_pool(name="gpool", bufs=2) as gpool,      # per-side working tiles
            tc.tile_pool(name="bpool", bufs=2) as bpool,      # batch-phase tiles
            tc.tile_pool(name="b1pool", bufs=1) as b1pool,    # embT/hT
            tc.tile_pool(name="cpool", bufs=1) as cpool,
            tc.tile_pool(name="psum", bufs=2, space="PSUM") as pp,
            tc.tile_pool(name="psum2", bufs=2, space="PSUM") as pp2,
        ):
            ident = cpool.tile([128, 128], bf16)
            make_identity(nc, ident[:])

            # ---- hoisted loads for BOTH sides: small tensors first so early
            # compute isn't head-of-line blocked behind the big grid DMAs
            Fall, maskneg, dstT, wa_s, wa_d, bslot, wsm = {}, {}, {}, {}, {}, {}, {}
            FT = {}
            for s in ("u", "i"):
                FT[s] = fpool.tile([EMB, SL, 128], bf16, tag=f"FT{s}", name=f"FT{s}")
                Fall[s] = fpool.tile([128, SL, EMB], bf16, tag=f"Fall{s}", name=f"Fall{s}")
            c1h = int(CW[4])
            c2h = int(CW[8])
            prev_c = 0
            for cb in (int(CW[1]), int(CW[2]), c1h):
                nc.sync.dma_start(
                    FT["u"][:, prev_c:cb, :].rearrange("p s d -> p (s d)"),
                    T["fgridT_u"][:, prev_c * 128:cb * 128])
                prev_c = cb
            prev_c = 0
            for cb in (int(CW[1]), int(CW[2]), c1h):
                nc.sync.dma_start(
                    FT["i"][:, prev_c:cb, :].rearrange("p s d -> p (s d)"),
                    T["fgridT_i"][:, prev_c * 128:cb * 128])
                prev_c = cb
            for cb0, cb1 in ((c1h, int(CW[6])), (int(CW[6]), c2h)):
                nc.sync.dma_start(
                    FT["u"][:, cb0:cb1, :].rearrange("p s d -> p (s d)"),
                    T["fgridT_u"][:, cb0 * 128:cb1 * 128])
            for s in ("u", "i"):
                maskneg[s] = lpool.tile([128, SL], f32, tag=f"maskneg{s}", name=f"maskneg{s}")
                nc.sync.dma_start(maskneg[s][:], T[f"maskneg_{s}"][:])
                dstT[s] = lpool.tile([128, NW, EMB], bf16, tag=f"dstT{s}", name=f"dstT{s}")
                nc.sync.dma_start(
                    dstT[s][:], T[f"dst_tab_{s}"][:].rearrange("(n p) d -> p n d", p=128))
                wa_s[s] = lpool.tile([EMB, 1], bf16, tag=f"was{s}", name=f"was{s}")
                nc.sync.dma_start(wa_s[s][:], T[f"wa_src_{s}"][:])
                wa_d[s] = lpool.tile([128, EMB], bf16, tag=f"wad{s}", name=f"wad{s}")
                nc.sync.dma_start(wa_d[s][:], T[f"wa_dst_{s}"][:])
                bslot[s] = lpool.tile([128, BC // 16], i16, tag=f"bslot{s}", name=f"bslot{s}")
                nc.sync.dma_start(bslot[s][:], T[f"bslot_{s}"][:])
                wsm[s] = {}
                wsn = lpool.tile([128, EMB], bf16, tag=f"wsn{s}", name=f"wsn{s}")
                nc.sync.dma_start(wsn[:EMB, :], T[f"WsT_{s}"][:])
                nc.sync.dma_start(wsn[EMB:, :], T[f"WnT_{s}"][:])
                wsm[s]["wsn"] = wsn
                for nm in ("WfcS", "WfcN"):
                    wsm[s][nm] = lpool.tile([EMB, EMB], bf16, tag=f"{nm}{s}", name=f"{nm}{s}")
                    nc.sync.dma_start(wsm[s][nm][:], T[f"{nm}_{s}"][:])
                for nm in ("bs", "bn"):
                    wsm[s][nm] = lpool.tile([EMB, 1], f32, tag=f"{nm}{s}", name=f"{nm}{s}")
                    nc.sync.dma_start(wsm[s][nm][:], T[f"{nm}_{s}"][:])
            for s in ("u", "i"):
                for h in range(4):
                    c0, c1 = int(CW[4 * h]), int(CW[4 * (h + 1)])
                    if not (h == 0 or (s == "u" and h == 1)):
                        if s == "i" and h == 1:
                            for cb0, cb1 in ((c0, int(CW[6])), (int(CW[6]), c1)):
                                nc.sync.dma_start(
                                    FT[s][:, cb0:cb1, :].rearrange("p s d -> p (s d)"),
                                    T[f"fgridT_{s}"][:, cb0 * 128:cb1 * 128])
                        elif h == 2:
                            for cb0, cb1 in ((c0, int(CW[10])), (int(CW[10]), c1)):
                                nc.sync.dma_start(
                                    FT[s][:, cb0:cb1, :].rearrange("p s d -> p (s d)"),
                                    T[f"fgridT_{s}"][:, cb0 * 128:cb1 * 128])
                        else:
                            nc.sync.dma_start(
                                FT[s][:, c0:c1, :].rearrange("p s d -> p (s d)"),
                                T[f"fgridT_{s}"][:, c0 * 128:c1 * 128])
                    nc.sync.dma_start(
                        Fall[s][:, c0:c1, :].rearrange("p s d -> p (s d)"),
                        T[f"fgrid_{s}"][:, c0 * EMB:c1 * EMB])

            for si, s in enumerate(("u", "i")):
                # ---- s_dst per grid lane: [128, NW]
                sd_t = gpool.tile([128, NW, EMB], bf16, tag="sdt")
                nc.gpsimd.tensor_tensor(
                    out=sd_t[:], in0=dstT[s][:],
                    in1=wa_d[s][:].unsqueeze(1).to_broadcast([128, NW, EMB]),
                    op=Alu.mult)
                sdst = gpool.tile([128, NW], f32, tag="sdst")
                nc.vector.tensor_reduce(
                    out=sdst[:], in_=sd_t[:],
                    axis=mybir.AxisListType.X, op=Alu.add)

                # ---- whole-grid edge scores ss = sum_d F * wa (two halves)
                ss = gpool.tile([128, SL], f32, tag="ss")
                bounds = [int(CW[1]), int(CW[2]), int(CW[3]), int(CW[4]),
                          int(CW[6]), int(CW[8]), int(CW[12]), SL]
                prev = 0
                for c1 in bounds:
                    c0, c1 = prev, c1
                    prev = c1
                    pss = pp.tile([128, c1 - c0], f32, tag="pss", name="pss")
                    for c in range(c0, c1):
                        nc.tensor.matmul(
                            pss[:, c - c0:c - c0 + 1], FT[s][:, c, :],
                            wa_s[s][:], start=True, stop=True)
                    nc.scalar.copy(out=ss[:, c0:c1], in_=pss[:])

                # ---- e = lrelu(ss + sdst) per window on ACT; mask; exp+den
                lr = gpool.tile([128, SL], f32, tag="lr")
                for w in range(NW):
                    cw, k = int(CW[w]), KS[w]
                    nc.scalar.activation(
                        lr[:, cw:cw + k], ss[:, cw:cw + k], Act.Lrelu,
                        bias=sdst[:, w:w + 1], alpha=0.01)
                lrm = gpool.tile([128, SL], f32, tag="lrm")
                for h in range(2):
                    c0, c1 = (0, SH) if h == 0 else (SH, SL)
                    nc.gpsimd.tensor_tensor(
                        out=lrm[:, c0:c1], in0=lr[:, c0:c1],
                        in1=maskneg[s][:, c0:c1], op=Alu.add)
                ex = gpool.tile([128, SL], f32, tag="ex")
                den = gpool.tile([128, NW], f32, tag="den")
                for w in range(NW):
                    cw, k = int(CW[w]), KS[w]
                    nc.scalar.activation(
                        ex[:, cw:cw + k], lrm[:, cw:cw + k], Act.Exp,
                        accum_out=den[:, w:w + 1])

                # ---- guarded reciprocal
                nc.vector.tensor_scalar_max(out=den[:], in0=den[:], scalar1=F32MIN)
                invd = gpool.tile([128, NW], f32, tag="invd")
                nc.vector.reciprocal(invd[:], den[:])

                # ---- unnormalized weighted sums, transposed layout
                fprod = fpool.tile([128, EMB, SL], bf16, tag="fprod")
                SPLIT = int(CW[8])
                for h in range(2):
                    c0, c1 = (0, SPLIT) if h == 0 else (SPLIT, SL)
                    eng = nc.vector if h == 0 else nc.gpsimd
                    eng.tensor_tensor(
                        out=fprod[:, :, c0:c1].rearrange("p d s -> p s d"),
                        in0=Fall[s][:, c0:c1, :],
                        in1=ex[:, c0:c1].unsqueeze(2).to_broadcast(
                            [128, c1 - c0, EMB]),
                        op=Alu.mult)
                hgrid = gpool.tile([128, NW, EMB], bf16, tag="hgrid")
                with nc.allow_low_precision("bf16 segment sums, <=24 addends"):
                    for w in range(NW):
                        cw, k = int(CW[w]), KS[w]
                        nc.vector.tensor_reduce(
                            out=hgrid[:, w, :], in_=fprod[:, :, cw:cw + k],
                            axis=mybir.AxisListType.X, op=Alu.add)
                hgridn = gpool.tile([128, NW, EMB], bf16, tag="hgridn")
                for w in range(NW):
                    nc.gpsimd.tensor_scalar_mul(
                        out=hgridn[:, w, :], in0=hgrid[:, w, :],
                        scalar1=invd[:, w:w + 1])

                # ---- fused [emb | h] scratch in DRAM
                nc.sync.dma_start(
                    T[f"scratch_{s}"][:, :EMB].rearrange("(n p) d -> p n d", p=128),
                    dstT[s][:])
                for rh in range(4):
                    r0, r1 = rh * (GPAD // 4), (rh + 1) * (GPAD // 4)
                    nc.sync.dma_start(
                        T[f"scratch_{s}"][r0:r1, EMB:].rearrange(
                            "(n p) d -> p n d", p=128),
                        hgridn[:, r0 // 128:r1 // 128, :])

            for si, s in enumerate(("u", "i")):
                # ---- batch gather (two 1024-idx calls on dedicated queues)
                cat = bpool.tile([128, BC // 128, 2 * EMB], bf16, tag="cat")
                for half in range(2):
                    nc.gpsimd.dma_gather(
                        cat[:, half * 8:(half + 1) * 8, :],
                        T[f"scratch_{s}"][:],
                        bslot[s][:, half * 64:(half + 1) * 64],
                        1024, 1024, 2 * EMB, single_packet=True,
                        queue_num=si * 2 + half)
                # fused [emb|h] transpose: one [128,128] PE transpose per chunk;
                # rows 0..63 of catT are emb dims, 64..127 are h dims
                catT = b1pool.tile([128, BC], bf16, tag="catT")
                for t in range(BC // 128):
                    pt_ = pp.tile([128, 128], bf16, tag="pt")
                    nc.tensor.transpose(pt_[:], cat[:, t, :], ident[:])
                    dst_sl = catT[:, t * 128:(t + 1) * 128]
                    if si == 0:
                        nc.scalar.copy(out=dst_sl, in_=pt_[:])
                    else:
                        nc.vector.tensor_copy(out=dst_sl, in_=pt_[:])

                # ---- batch MLP: sf/nb linears + relu, then fc + relu
                CHK = 512
                for q in range(BC // CHK):
                    sl_ = slice(q * CHK, (q + 1) * CHK)
                    pcomb = pp2.tile([128, CHK], f32, tag="pcomb")
                    nc.tensor.matmul(pcomb[:EMB, :], wsm[s]["wsn"][:EMB, :], catT[:EMB, sl_], start=True, stop=True)
                    nc.tensor.matmul(pcomb[EMB:, :], wsm[s]["wsn"][EMB:, :], catT[EMB:, sl_], start=True, stop=True)
                    sfr = bpool.tile([EMB, CHK], bf16, tag="sfr")
                    nc.scalar.activation(sfr[:], pcomb[:EMB, :], Act.Relu, bias=wsm[s]["bs"][:])
                    nbr = bpool.tile([EMB, CHK], bf16, tag="nbr")
                    nc.scalar.activation(nbr[:], pcomb[EMB:, :], Act.Relu, bias=wsm[s]["bn"][:])
                    pv = pp2.tile([EMB, CHK], f32, tag="pv")
                    nc.tensor.matmul(pv[:], wsm[s]["WfcS"][:], sfr[:], start=True, stop=False)
                    nc.tensor.matmul(pv[:], wsm[s]["WfcN"][:], nbr[:], start=False, stop=True)
                    ov = bpool.tile([EMB, CHK], f32, tag="ov")
                    nc.vector.tensor_scalar_max(out=ov[:], in0=pv[:], scalar1=0.0)
                    nc.sync.dma_start(T[f"outT_{s}"][:, sl_], ov[:])

    nc.compile()
    return nc


# ------------------------------------------------------------------ assembly

def _assemble(results):
    out = np.empty((B, 2 * EMB), np.float32)
    for c, r in enumerate(results):
        out[c * BC:(c + 1) * BC, :EMB] = r["outT_u"].T
        out[c * BC:(c + 1) * BC, EMB:] = r["outT_i"].T
    return out


def build_all(inputs):
    cfg, per_core = _prep_all(inputs)
    nc = _build_nc(cfg)
    return nc, per_core


def kernel(**inputs) -> np.ndarray:
    from concourse.bass_utils import run_bass_kernel_spmd
    nc, per_core = build_all(inputs)
    res = run_bass_kernel_spmd(nc, per_core, core_ids=list(range(NCORES)))
    return _assemble(res.results)



# revision 12
# speedup vs baseline: 1.5505x; 1.5505x over previous
"""MetaGAT Trainium2 kernel (8 NeuronCores, SPMD) — v2.

Strategy (batch-filtered slot grid, host-staged dense edge data):
  Each core takes a 2048-slice of the batch; the host selects edges whose
  destination is in the slice's id set (~20K of 2M per core per side) and
  builds a degree-bucketed slot grid (dst -> lane, edge -> grid column).
  Host stages the grid's source embeddings DENSELY in two layouts:
    - fgridW bf16 [128, 64(d), SL(s)]  (s innermost -> DVE 2x mode)
    - fgridT fp8  [64(d), SL*128]      (score path; padding columns hold
      -240*sign(wa) so masked slots score ~-1000 -> exp ~0; no mask tensor)
  All model arithmetic on-device:
    - edge scores: per-slot PE matmuls s = F_c^T wa_src (+ dst-bias via a
      second accumulating matmul against dst_tabT window blocks)
    - lrelu on DVE (scalar_tensor_tensor max(x, .01x)) -> exp on ACT
    - per-window softmax den on Pool, recip on DVE, ex normalized in-place
    - fprod = fgridW * exn (DVE 2x), window sums accumulated on PE via
      stationary-identity matmuls into PSUM, normalized (already via exn),
      transposed per window on PE -> hT [64, 2048]
    - batch MLP on PE/ACT from dst_tabT (self path) and hT (neighbor path)
  Outputs are [64, 2048] bf16 in SLOT order; the host gathers slot->batch
  (duplicates share a slot) and emits the [16384, 128] f32 result.
"""
import numpy as np
import ml_dtypes

EMB = 64
NNODE = 200000
NCORES = 8
B = 16384
BC = B // NCORES          # 2048 batch rows per core
GPAD = BC                 # grid slots
NW = GPAD // 128          # 16 windows of 128 dst lanes
F32MIN = 1e-30
FP8PAD = -240.0           # max-magnitude e4m3 value for padding columns


# ----------------------------------------------------------------- host prep

def _prep_core_side(bat_c, src_ids, dst_ids):
    """Pure index bookkeeping for one (core, side): select + grid-order edges."""
    uniq, inv = np.unique(bat_c, return_inverse=True)
    G = uniq.size
    lut = np.full(NNODE, -1, np.int32)
    lut[uniq] = np.arange(G, dtype=np.int32)
    eg = lut[dst_ids]
    m = eg >= 0
    es = src_ids[m].astype(np.int64)
    eg = eg[m].astype(np.int64)
    deg = np.bincount(eg, minlength=G)
    order = np.argsort(-deg, kind="stable")          # grid rank -> uniq idx
    pos = np.empty(G, np.int64)
    pos[order] = np.arange(G)
    deg_r = deg[order]                               # degree by rank (desc)
    ep = pos[eg]                                     # edge -> grid rank
    eo = np.argsort(ep, kind="stable")
    es_s = es[eo]                                    # edge -> src node id
    ep_s = ep[eo]
    starts = np.zeros(G + 1, np.int64)
    np.cumsum(deg_r, out=starts[1:])
    ii = np.arange(es_s.size) - starts[ep_s]         # slot index within dst
    bslot = pos[inv]                                 # batch row -> grid rank
    return dict(G=G, uniq=uniq, order=order, deg_r=deg_r, es_s=es_s,
                ep_s=ep_s, ii=ii, bslot=bslot)


def _chunks(KS, n):
    """Split window indices 0..NW-1 into n groups with ~equal column counts."""
    tot = sum(KS)
    groups, cur, acc = [], [], 0
    for w, k in enumerate(KS):
        cur.append(w)
        acc += k
        if acc >= tot * (len(groups) + 1) / n and len(groups) < n - 1:
            groups.append(cur)
            cur = []
    groups.append(cur)
    return groups


def _streams(pc, KS, emb_src, emb_dst, wa_src):
    """Build device arrays for one (core, side) under the common schedule KS."""
    SL = sum(KS)
    CW = np.concatenate([[0], np.cumsum(KS)]).astype(np.int64)
    G = pc["G"]
    lane = pc["ep_s"] % 128
    win = pc["ep_s"] // 128
    col = CW[win] + pc["ii"]
    F = np.zeros((128, SL, EMB), np.float32)
    F[lane, col, :] = emb_src[pc["es_s"]]
    # fgridW: [128, 64, SL] chunk-major over W-chunks (s innermost)
    FW = F.transpose(0, 2, 1)                        # [128, 64, SL]
    wch = _chunks(KS, 4)
    parts = []
    for ws in wch:
        c0, c1 = int(CW[ws[0]]), int(CW[ws[-1] + 1])
        parts.append(np.ascontiguousarray(FW[:, :, c0:c1]).reshape(128, -1))
    fgridW = np.concatenate(parts, axis=1).astype(ml_dtypes.bfloat16)
    # fgridT: [64, SL*128] fp8, padding columns = FP8PAD*sign(wa)
    FT = np.ascontiguousarray(F.transpose(2, 1, 0))  # [64, SL, 128]
    valid = np.zeros((128, SL), bool)
    valid[lane, col] = True
    padcol = (FP8PAD * np.where(wa_src >= 0, 1.0, -1.0)).astype(np.float32)
    inv_l, inv_c = np.nonzero(~valid)
    FT[:, inv_c, inv_l] = padcol[:, None]
    tch = _chunks(KS, 2)
    parts = []
    for ws in tch:
        c0, c1 = int(CW[ws[0]]), int(CW[ws[-1] + 1])
        parts.append(np.ascontiguousarray(FT[:, c0:c1, :]).reshape(64, -1))
    fgridT = np.concatenate(parts, axis=1).astype(ml_dtypes.float8_e4m3)
    # dst_tabT: [64, GPAD] bf16, slot-major dst embeddings (zeros padding)
    dstT = np.zeros((EMB, GPAD), np.float32)
    dstT[:, :G] = emb_dst[pc["uniq"][pc["order"]]].T
    dst_tabT = dstT.astype(ml_dtypes.bfloat16)
    return dict(fgridW=fgridW, fgridT=fgridT, dst_tabT=dst_tabT), pc["bslot"]


def _prep_all(inputs):
    u = np.asarray(inputs["u"]).astype(np.int64)
    i_ = np.asarray(inputs["i"]).astype(np.int64)
    sides = {
        "u": dict(bat=u, src=np.asarray(inputs["src_iu"]).astype(np.int64),
                  dst=np.asarray(inputs["dst_iu"]).astype(np.int64),
                  emb_src=np.asarray(inputs["item_emb"], np.float32),
                  emb_dst=np.asarray(inputs["user_emb"], np.float32)),
        "i": dict(bat=i_, src=np.asarray(inputs["src_ui"]).astype(np.int64),
                  dst=np.asarray(inputs["dst_ui"]).astype(np.int64),
                  emb_src=np.asarray(inputs["user_emb"], np.float32),
                  emb_dst=np.asarray(inputs["item_emb"], np.float32)),
    }
    pcs = {s: [_prep_core_side(sides[s]["bat"][c * BC:(c + 1) * BC],
                               sides[s]["src"], sides[s]["dst"])
               for c in range(NCORES)] for s in sides}
    # common window schedule across cores & sides
    KS = []
    for w in range(NW):
        k = 1
        for s in pcs:
            for pc in pcs[s]:
                if w * 128 < pc["G"]:
                    k = max(k, int(pc["deg_r"][w * 128]))
        KS.append(k)
    # staged weights (identical on every core)
    wa = {s: np.asarray(inputs[f"Wa_{s}"], np.float32) for s in ("u", "i")}
    wblob = []      # bf16 blob [64, ...]: per side WsT|WnT|WfcS|WfcN|wa_dst
    for s in ("u", "i"):
        Ws = np.asarray(inputs[f"Ws_{s}"], np.float32)
        Wn = np.asarray(inputs[f"Wn_{s}"], np.float32)
        Wfc = np.asarray(inputs[f"Wfc_{s}"], np.float32)
        wblob += [Ws.T, Wn.T, np.ascontiguousarray(Wfc[:, :EMB].T),
                  np.ascontiguousarray(Wfc[:, EMB:].T),
                  wa[s][EMB:].reshape(EMB, 1)]
    wblob = np.concatenate(wblob, axis=1).astype(ml_dtypes.bfloat16)
    bblob = np.stack([np.asarray(inputs["bs_u"], np.float32),
                      np.asarray(inputs["bn_u"], np.float32),
                      np.asarray(inputs["bs_i"], np.float32),
                      np.asarray(inputs["bn_i"], np.float32)], axis=1)
    ablob = np.stack([wa["u"][:EMB], wa["i"][:EMB]], axis=1).astype(
        ml_dtypes.float8_e4m3)
    per_core, bslots = [], []
    for c in range(NCORES):
        d = dict(wblob=wblob, bblob=np.ascontiguousarray(bblob),
                 ablob=np.ascontiguousarray(ablob))
        bs = {}
        for s in ("u", "i"):
            st, bslot = _streams(pcs[s][c], KS, sides[s]["emb_src"],
                                 sides[s]["emb_dst"], wa[s][:EMB])
            for k, v in st.items():
                d[f"{k}_{s}"] = v
            bs[s] = bslot
        per_core.append(d)
        bslots.append(bs)
    cfg = dict(KS=tuple(KS))
    return cfg, per_core, bslots


# ------------------------------------------------------------- device kernel

def _build_nc(cfg):
    import concourse.bacc as bacc
    import concourse.mybir as mybir
    import concourse.tile as tile
    from concourse.masks import make_identity

    f32 = mybir.dt.float32
    bf16 = mybir.dt.bfloat16
    fp8 = mybir.dt.float8e4
    KS = cfg["KS"]
    SL = sum(KS)
    CW = np.concatenate([[0], np.cumsum(KS)]).astype(int)
    wch = _chunks(KS, 4)          # fgridW chunks (window groups)
    tch = _chunks(KS, 2)          # fgridT chunks
    Alu = mybir.AluOpType
    Act = mybir.ActivationFunctionType
    CHK = 512                     # MLP column chunk
    NQ = GPAD // CHK              # 4 hT quarters / MLP chunks

    def t_bounds(ws):
        return int(CW[ws[0]]), int(CW[ws[-1] + 1])

    def t_sz(ws):
        c0, c1 = t_bounds(ws)
        return c1 - c0

    nc = bacc.Bacc("TRN2", num_swdge_queues=1)
    T = {}
    for s in ("u", "i"):
        T[f"fgridW_{s}"] = nc.dram_tensor(f"fgridW_{s}", [128, SL * EMB], bf16, kind="ExternalInput")
        T[f"fgridT_{s}"] = nc.dram_tensor(f"fgridT_{s}", [EMB, SL * 128], fp8, kind="ExternalInput")
        T[f"dst_tabT_{s}"] = nc.dram_tensor(f"dst_tabT_{s}", [EMB, GPAD], bf16, kind="ExternalInput")
        T[f"outT_{s}"] = nc.dram_tensor(f"outT_{s}", [EMB, GPAD], bf16, kind="ExternalOutput")
    T["wblob"] = nc.dram_tensor("wblob", [EMB, 2 * (4 * EMB + 1)], bf16, kind="ExternalInput")
    T["bblob"] = nc.dram_tensor("bblob", [EMB, 4], f32, kind="ExternalInput")
    T["ablob"] = nc.dram_tensor("ablob", [EMB, 2], fp8, kind="ExternalInput")

    with tile.TileContext(nc) as tc:
        with (
            tc.tile_pool(name="gpool", bufs=1) as gpool,     # big grids
            tc.tile_pool(name="lpool", bufs=1) as lpool,     # small loads
            tc.tile_pool(name="wpool", bufs=2) as wpool,     # per-side work
            tc.tile_pool(name="psA", bufs=2, space="PSUM") as psA,    # scores
            tc.tile_pool(name="psB", bufs=1, space="PSUM") as psB,    # hgrid
            tc.tile_pool(name="psC", bufs=2, space="PSUM") as psC,    # hT quarters
            tc.tile_pool(name="psD", bufs=1, space="PSUM") as psD,    # MLP
        ):
            ident = lpool.tile([128, 128], bf16, name="ident")
            make_identity(nc, ident[:])

            # ---- hoisted DMAs, interleaved by need (per-chunk tiles so each
            # DMA is 128 contiguous row descriptors)
            FW, FT, DT = {}, {}, {}
            for s in ("u", "i"):
                FW[s] = [gpool.tile([128, EMB, t_sz(wch[ci])], bf16,
                                    tag=f"FW{s}{ci}", name=f"FW{s}{ci}")
                         for ci in range(4)]
                FT[s] = [gpool.tile([EMB, t_sz(tch[ci]), 128], fp8,
                                    tag=f"FT{s}{ci}", name=f"FT{s}{ci}")
                         for ci in range(2)]
                DT[s] = gpool.tile([EMB, GPAD], bf16, tag=f"DT{s}", name=f"DT{s}")
            wblob = lpool.tile([EMB, 2 * (4 * EMB + 1)], bf16, name="wblob")
            bblob = lpool.tile([EMB, 4], f32, name="bblob")
            ablob = lpool.tile([EMB, 2], fp8, name="ablob")

            def dma_ft(s, ci):
                c0, c1 = t_bounds(tch[ci])
                nc.sync.dma_start(
                    FT[s][ci][:].rearrange("p s l -> p (s l)"),
                    T[f"fgridT_{s}"][:, c0 * 128:c1 * 128])

            def dma_fw(s, ci):
                c0, c1 = t_bounds(wch[ci])
                nc.sync.dma_start(
                    FW[s][ci][:].rearrange("p d s -> p (d s)"),
                    T[f"fgridW_{s}"][:, c0 * EMB:c1 * EMB])

            # order: score inputs for u first, then u aggregation data
            # interleaved with i score inputs, then i aggregation data.
            nc.sync.dma_start(ablob[:], T["ablob"][:])
            nc.sync.dma_start(wblob[:], T["wblob"][:])
            nc.sync.dma_start(bblob[:], T["bblob"][:])
            dma_ft("u", 0)
            nc.sync.dma_start(DT["u"][:], T["dst_tabT_u"][:])
            dma_ft("u", 1)
            dma_fw("u", 0)
            dma_fw("u", 1)
            dma_ft("i", 0)
            dma_fw("u", 2)
            dma_ft("i", 1)
            nc.sync.dma_start(DT["i"][:], T["dst_tabT_i"][:])
            dma_fw("u", 3)
            for ci in range(4):
                dma_fw("i", ci)

            WOFF = 4 * EMB + 1
            wa_dst = {s: wblob[:, si * WOFF + 4 * EMB:si * WOFF + 4 * EMB + 1]
                      for si, s in enumerate(("u", "i"))}
            wa_src = {s: ablob[:, si:si + 1] for si, s in enumerate(("u", "i"))}
            wmat = {s: {nm: wblob[:, si * WOFF + j * EMB:si * WOFF + (j + 1) * EMB]
                        for j, nm in enumerate(("WsT", "WnT", "WfcS", "WfcN"))}
                    for si, s in enumerate(("u", "i"))}
            bias = {"u": {"bs": bblob[:, 0:1], "bn": bblob[:, 1:2]},
                    "i": {"bs": bblob[:, 2:3], "bn": bblob[:, 3:4]}}

            st = {}
            for s in ("u", "i"):
                st[s] = dict(
                    pss=psA.tile([128, SL], f32, tag="pss", name=f"pss{s}"),
                    lr=wpool.tile([128, SL], bf16, tag="lr", name=f"lr{s}"),
                    lr01=wpool.tile([128, SL], bf16, tag="lr01", name=f"lr01{s}"),
                    ex=wpool.tile([128, SL], bf16, tag="ex", name=f"ex{s}"),
                    exn=wpool.tile([128, SL], bf16, tag="exn", name=f"exn{s}"),
                    den=wpool.tile([128, NW], f32, tag="den", name=f"den{s}"),
                    rden=wpool.tile([128, NW], f32, tag="rden", name=f"rden{s}"),
                    fprod=gpool.tile([128, EMB, SL], bf16, tag=f"fp{s}", name=f"fp{s}"),
                    hgrid=psB.tile([128, NW * EMB], f32, tag="hgrid", name=f"hg{s}"),
                    hgridn=wpool.tile([128, NW, EMB], bf16, tag="hgn", name=f"hgn{s}"),
                    hT=wpool.tile([EMB, GPAD], bf16, tag="hT", name=f"hT{s}"),
                    ov=wpool.tile([EMB, GPAD], bf16, tag="ov", name=f"ov{s}"),
                )

            def scores(s, ci):
                """Per-slot PE matmuls for fgridT chunk ci: s_src + s_dst."""
                d = st[s]
                t0, _ = t_bounds(tch[ci])
                for w in tch[ci]:
                    cw, k = int(CW[w]), KS[w]
                    dstw = DT[s][:, w * 128:(w + 1) * 128]
                    for j in range(k):
                        c = cw + j
                        nc.tensor.matmul(d["pss"][:, c:c + 1],
                                         FT[s][ci][:, c - t0, :],
                                         wa_src[s], start=True, stop=False)
                        nc.tensor.matmul(d["pss"][:, c:c + 1], dstw,
                                         wa_dst[s], start=False, stop=True)

            def softmax_head(s, ci):
                """lrelu + exp for fgridT chunk ci. HW allows only one PSUM
                input per DVE op, so ACT stages 0.01*pss into SBUF first."""
                d = st[s]
                c0, c1 = t_bounds(tch[ci])
                nc.scalar.activation(d["lr01"][:, c0:c1], d["pss"][:, c0:c1],
                                     Act.Copy, scale=0.01)
                nc.vector.tensor_tensor(
                    out=d["lr"][:, c0:c1], in0=d["lr01"][:, c0:c1],
                    in1=d["pss"][:, c0:c1], op=Alu.max)
                nc.scalar.activation(d["ex"][:, c0:c1], d["lr"][:, c0:c1], Act.Exp)

            def den_norm(s, ci):
                """Per-window den (Pool), recip (DVE), exn (DVE) for chunk ci."""
                d = st[s]
                ws = tch[ci]
                for w in ws:
                    cw, k = int(CW[w]), KS[w]
                    nc.vector.tensor_reduce(
                        out=d["den"][:, w:w + 1], in_=d["ex"][:, cw:cw + k],
                        axis=mybir.AxisListType.X, op=Alu.add)
                w0, w1 = ws[0], ws[-1] + 1
                nc.vector.tensor_scalar_max(out=d["den"][:, w0:w1],
                                            in0=d["den"][:, w0:w1], scalar1=F32MIN)
                nc.vector.reciprocal(d["rden"][:, w0:w1], d["den"][:, w0:w1])
                for w in ws:
                    cw, k = int(CW[w]), KS[w]
                    nc.vector.tensor_scalar_mul(
                        out=d["exn"][:, cw:cw + k], in0=d["ex"][:, cw:cw + k],
                        scalar1=d["rden"][:, w:w + 1])

            def aggregate(s, ci):
                """fprod mult (DVE 2x) + PE window accumulation for W-chunk ci."""
                d = st[s]
                c0, c1 = t_bounds(wch[ci])
                nc.vector.tensor_tensor(
                    out=d["fprod"][:, :, c0:c1], in0=FW[s][ci][:],
                    in1=d["exn"][:, c0:c1].unsqueeze(1).to_broadcast(
                        [128, EMB, c1 - c0]),
                    op=Alu.mult)
                for w in wch[ci]:
                    cw, k = int(CW[w]), KS[w]
                    for j in range(k):
                        nc.tensor.matmul(
                            d["hgrid"][:, w * EMB:(w + 1) * EMB], ident[:],
                            d["fprod"][:, :, cw + j], start=(j == 0),
                            stop=(j == k - 1))

            def head_out(s, q):
                """Windows 4q..4q+3: drain hgrid (Pool copy), transpose (PE),
                drain hT quarter (ACT)."""
                d = st[s]
                hq = psC.tile([EMB, CHK], bf16, tag="hq", name=f"hq{s}{q}")
                nc.scalar.copy(
                    out=d["hgridn"][:, 4 * q:4 * q + 4, :].rearrange(
                        "p w d -> p (w d)"),
                    in_=d["hgrid"][:, 4 * q * EMB:(4 * q + 4) * EMB])
                for w in range(4 * q, 4 * q + 4):
                    nc.tensor.transpose(hq[:, (w % 4) * 128:(w % 4 + 1) * 128],
                                        d["hgridn"][:, w, :], ident[:])
                nc.scalar.copy(out=d["hT"][:, q * CHK:(q + 1) * CHK], in_=hq[:])

            def mlp(s, q):
                d = st[s]
                sl_ = slice(q * CHK, (q + 1) * CHK)
                pc_ = psD.tile([128, CHK], f32, tag="pc", name=f"pc{s}{q}")
                nc.tensor.matmul(pc_[:EMB, :], wmat[s]["WsT"], DT[s][:, sl_],
                                 start=True, stop=True)
                nc.tensor.matmul(pc_[EMB:, :], wmat[s]["WnT"], d["hT"][:, sl_],
                                 start=True, stop=True)
                sfr = wpool.tile([EMB, CHK], bf16, tag="sfr", name=f"sfr{s}{q}")
                nbr = wpool.tile([EMB, CHK], bf16, tag="nbr", name=f"nbr{s}{q}")
                nc.scalar.activation(sfr[:], pc_[:EMB, :], Act.Relu, bias=bias[s]["bs"])
                nc.scalar.activation(nbr[:], pc_[EMB:, :], Act.Relu, bias=bias[s]["bn"])
                pv = psD.tile([EMB, CHK], f32, tag="pv", name=f"pv{s}{q}")
                nc.tensor.matmul(pv[:], wmat[s]["WfcS"], sfr[:], start=True, stop=False)
                nc.tensor.matmul(pv[:], wmat[s]["WfcN"], nbr[:], start=False, stop=True)
                nc.vector.tensor_scalar_max(out=d["ov"][:, sl_], in0=pv[:], scalar1=0.0)

            # ---- pipeline: u score path, u aggregation; i interleaved behind
            for ci in range(2):
                scores("u", ci)
                softmax_head("u", ci)
                den_norm("u", ci)
            for ci in range(4):
                aggregate("u", ci)
            for ci in range(2):
                scores("i", ci)
                softmax_head("i", ci)
                den_norm("i", ci)
            for q in range(NQ):
                head_out("u", q)
            for ci in range(4):
                aggregate("i", ci)
            for q in range(NQ):
                mlp("u", q)
            for q in range(NQ):
                head_out("i", q)
            for q in range(NQ):
                mlp("i", q)
            nc.sync.dma_start(T["outT_u"][:], st["u"]["ov"][:])
            nc.sync.dma_start(T["outT_i"][:], st["i"]["ov"][:])

    nc.compile()
    return nc


# ------------------------------------------------------------------ assembly

def _assemble(results, bslots):
    out = np.empty((B, 2 * EMB), np.float32)
    for c, r in enumerate(results):
        for s, off in (("u", 0), ("i", EMB)):
            full = np.asarray(r[f"outT_{s}"]).astype(np.float32)  # [64, GPAD]
            out[c * BC:(c + 1) * BC, off:off + EMB] = full[:, bslots[c][s]].T
    return out


def build_all(inputs):
    cfg, per_core, bslots = _prep_all(inputs)
    nc = _build_nc(cfg)
    return nc, per_core, bslots


def kernel(**inputs) -> np.ndarray:
    from concourse.bass_utils import run_bass_kernel_spmd
    nc, per_core, bslots = build_all(inputs)
    res = run_bass_kernel_spmd(nc, per_core, core_ids=list(range(NCORES)))
    return _assemble(res.results, bslots)
